# revision 21
# baseline (speedup 1.0000x reference)
"""Trainium2 Bass kernel: 2-layer GCN (GCNConv -> ReLU -> GCNConv -> Linear).

Strategy (8 NeuronCores, SPMD, 3 launches with host-side exchange):
  - Destination-node sharding with degree-sorted serpentine assignment.
  - The host reorders activation tables into *edge order* between launches
    (pure data movement / dtype casts), so each launch streams its operands
    sequentially at HWDGE line rate -- no on-device gather descriptors.
      L1: H1 = X @ W1    (transposed orientation: W1 stationary, node dim
                          streams in N=512 matmuls; emits H1^T, host detiles)
      L2: MP1 + bias + ReLU, @ (W2 Wp) (segment reduction via PE one-hot
                                        weight-slab matmuls over pre-ordered
                                        fp8 message chunks)
      L3: MP2 + bias                   (same geometry, F=128)
  - Segment reduction: edges (incl. self loops) sorted by destination; each
    chunk of 128 edge slots is one [128, F] message tile; a [128, M] slab
    block (lhsT, norm weights scattered at (slot, dst-lane)) contracts it
    into the destination rows of a PSUM tile.  Bias via a leading
    identity-x-bias matmul (start covers the full region).
  - Messages are fp8 e3m4 with per-table-row scales; the scale of each
    edge's source row is folded into that edge's slab weight (bf16).
"""

from contextlib import ExitStack
from dataclasses import dataclass, field

import numpy as np
import ml_dtypes

BF16 = ml_dtypes.bfloat16
FP8 = ml_dtypes.float8_e3m4
FP8_MAX = 14.0
FP32 = np.float32


# ---------------------------------------------------------------- config

@dataclass
class Cfg:
    N: int = 50000
    IN_DIM: int = 512
    HID: int = 256
    OUT: int = 128
    NCORES: int = 8
    GC: int = 64          # message chunks per DMA group
    TG: int = 8           # dest tiles per output DMA group
    NG1: int = 4          # L1: tiles per node group (N=512 streams)
    fp8_msg: bool = True
    fp8_x: bool = True
    full_slab_l2: bool = True   # L2: M=128 windows + fp8 slab -> FWL hides LDW

    ND: int = field(init=False)
    NTILES: int = field(init=False)
    NP: int = field(init=False)
    TROWS: int = field(init=False)
    G1: int = field(init=False)

    def __post_init__(self):
        self.ND = self.N // self.NCORES
        self.NTILES = (self.ND + 127) // 128
        self.NP = self.NTILES * 128
        self.TROWS = self.NCORES * self.NP
        self.G1 = -(-self.NTILES // self.NG1)


# ---------------------------------------------------------------- planner

class Plan:
    """Static (cross-core identical) geometry + per-core data arrays."""

    def __init__(self, cfg: Cfg, edge_index, edge_weight):
        self.cfg = cfg
        N, ND, NP, NT = cfg.N, cfg.ND, cfg.NP, cfg.NTILES
        NC = cfg.NCORES

        # --- gcn_norm with self loops (host: O(E) index/weight preprocessing)
        row = np.asarray(edge_index[0], np.int64)
        col = np.asarray(edge_index[1], np.int64)
        w = np.asarray(edge_weight, np.float64)
        deg = np.ones(N, np.float64)          # self-loop weight 1.0
        np.add.at(deg, col, w)
        dinv = np.where(deg > 0, 1.0 / np.sqrt(deg), 0.0)
        nrm = (dinv[row] * w * dinv[col]).astype(np.float32)

        # --- global degree-sorted serpentine node->(core, lane) assignment
        degi = np.bincount(col, minlength=N)
        ranks = np.argsort(-degi, kind="stable")    # rank r -> node
        r = np.arange(N)
        blk = r // NC
        corepos = np.where(blk % 2 == 0, r % NC, NC - 1 - (r % NC))
        lane_r = blk
        lane_global = np.empty(N, np.int64)        # node -> core*NP + lane
        lane_global[ranks] = corepos * NP + lane_r
        self.nodes = []                             # per core: lane -> node id
        for k in range(NC):
            nk = np.empty(ND, np.int64)
            sel = corepos == k
            nk[lane_r[sel]] = ranks[sel]
            self.nodes.append(nk)

        # --- edge stream incl. self loops, sorted by destination
        row_all = np.concatenate([row, np.arange(N, dtype=np.int64)])
        col_all = np.concatenate([col, np.arange(N, dtype=np.int64)])
        w_all = np.concatenate([nrm, (dinv * dinv).astype(np.float32)])

        src_t = lane_global[row_all]                # table row of the source
        dstg = lane_global[col_all]
        dst_core = dstg // NP
        dlane = dstg % NP
        dtile = dlane // 128
        dl = dlane - dtile * 128

        order = np.lexsort((dl, dtile, dst_core))
        sc = dst_core[order]
        st = dtile[order]
        sl = dl[order]
        ssrc = src_t[order]
        sw = w_all[order]

        key = sc * NT + st
        cnt = np.bincount(key, minlength=NC * NT).reshape(NC, NT)
        CH = (-(-cnt // 128)).max(axis=0)            # [NT] static chunks/tile
        self.base = np.concatenate([[0], np.cumsum(CH)]).astype(np.int64)
        self.tot = int(self.base[-1])

        seg_start = np.concatenate(
            [[0], np.cumsum(np.bincount(key, minlength=NC * NT))])[:-1]
        rank = np.arange(len(key)) - seg_start[key]
        chunk = self.base[st] + rank // 128          # static chunk id
        lanepos = rank % 128

        # --- cross-core chunk windows with legal matmul out bases (0/32/64)
        mn = np.full(self.tot, 128, np.int64)
        mx = np.full(self.tot, -1, np.int64)
        np.minimum.at(mn, chunk, sl)
        np.maximum.at(mx, chunk, sl)
        empty = mx < 0
        mn[empty] = 0
        mx[empty] = -1
        b32 = (mn // 32) * 32
        m32 = mx - b32 + 1
        b64 = (mn // 64) * 64
        m64 = mx - b64 + 1
        ok32 = (m32 <= 32) & (b32 <= 64)
        ok64 = m64 <= 64
        B = np.where(ok32, b32, np.where(ok64, b64, 0))
        M = np.where(ok32, m32, np.where(ok64, m64, mx + 1))
        M[empty] = 0
        B[empty] = 0
        self.cB = B
        self.cM = M
        self.slab_off = np.concatenate([[0], np.cumsum(M)])[:-1]
        self.SLAB = max(int(M.sum()), 1)

        # full-window layout (M=128 for every chunk): weights have 128
        # columns so the compiler enables FWL and LDWEIGHTS hides behind the
        # matmul stream
        self.SLAB2 = self.tot * 128

        # --- per-core arrays (slab values are built per launch: the fp8
        # per-row scale of each edge's source folds into its weight)
        self.midx = []    # slot -> table row, len tot*128
        self.edata = []   # (lanepos, slabcol, fullcol, weight f32, src row)
        for k in range(NC):
            m = sc == k
            idx = np.zeros(self.tot * 128, np.int64)
            idx[chunk[m] * 128 + lanepos[m]] = ssrc[m]
            self.midx.append(idx)
            self.edata.append((lanepos[m],
                               self.slab_off[chunk[m]] + sl[m] - B[chunk[m]],
                               chunk[m] * 128 + sl[m],
                               sw[m].astype(np.float32),
                               ssrc[m]))

    def build_slab(self, k, row_scale):
        lp, col, _, w, src = self.edata[k]
        slab = np.zeros((128, self.SLAB), np.float32)
        slab[lp, col] = w * row_scale[src]
        return slab.astype(BF16)

    def build_slab_full(self, k, row_scale):
        """fp8 full-window slab + the inverse of its global scale (alpha)."""
        lp, _, fcol, w, src = self.edata[k]
        vals = w * row_scale[src]
        alpha = max(float(np.abs(vals).max()), 1e-30) / FP8_MAX
        slab = np.zeros((128, self.SLAB2), np.float32)
        slab[lp, fcol] = vals / alpha
        return slab.astype(FP8), alpha


# ---------------------------------------------------------------- bass builders

def _build_l1(cfg: Cfg):
    import concourse.bacc as bacc
    import concourse.mybir as mybir
    import concourse.tile as tile

    dt = mybir.dt
    nc = bacc.Bacc(None, target_bir_lowering=False)
    KCH = cfg.IN_DIM // 128          # 4 contraction chunks
    OCH = cfg.HID // 128             # 2 output halves
    G1, NG1 = cfg.G1, cfg.NG1
    W = NG1 * 128                    # nodes per group (512)
    xdt = dt.float8e3 if cfg.fp8_x else dt.bfloat16
    xt = nc.dram_tensor("xt", [128, G1 * KCH * W], xdt, kind="ExternalInput")
    w1 = nc.dram_tensor("w1", [128, KCH * cfg.HID], dt.bfloat16,
                        kind="ExternalInput")
    # h1t[p, (g*OCH + o)*W + n] = H1[g*W + n, o*128 + p]
    h1t = nc.dram_tensor("h1t", [128, G1 * OCH * W], dt.bfloat16,
                         kind="ExternalOutput")

    with tile.TileContext(nc) as tc, ExitStack() as ctx:
        consts = ctx.enter_context(tc.tile_pool(name="consts", bufs=1))
        outs = ctx.enter_context(tc.tile_pool(name="outs", bufs=2))
        psum = ctx.enter_context(tc.tile_pool(name="psum", bufs=4, space="PSUM"))

        w1_sb = consts.tile([128, KCH * cfg.HID], dt.bfloat16, tag="w1")
        nc.sync.dma_start(w1_sb[:], w1[:])

        # preload every x group up front: the PE then streams without gaps
        # (staying busy keeps the HAM clock-gate at full rate)
        xgs = []
        for g in range(G1):
            xg_t = consts.tile([128, KCH * W], xdt, tag=f"xg{g}")
            nc.sync.dma_start(xg_t[:], xt[:, g * KCH * W: (g + 1) * KCH * W])
            xgs.append(xg_t)

        for g in range(G1):
            xg_t = xgs[g]
            o_g = outs.tile([128, OCH * W], dt.bfloat16)
            for o in range(OCH):
                ps = psum.tile([128, W], dt.float32)
                for c in range(KCH):
                    # lhsT = W1 chunk [128k, 128feat]; rhs = x^T [128k, W]
                    nc.tensor.matmul(
                        ps[:],
                        w1_sb[:, c * cfg.HID + o * 128: c * cfg.HID + (o + 1) * 128],
                        xg_t[:, c * W: (c + 1) * W],
                        start=(c == 0), stop=(c == KCH - 1),
                    )
                if o % 2 == 0:
                    nc.scalar.activation(o_g[:, o * W: (o + 1) * W], ps[:],
                                         mybir.ActivationFunctionType.Copy)
                else:
                    nc.vector.tensor_copy(o_g[:, o * W: (o + 1) * W], ps[:])
            nc.scalar.dma_start(h1t[:, g * OCH * W: (g + 1) * OCH * W], o_g[:])
    nc.finalize()
    return nc


def _build_mp(cfg: Cfg, plan: Plan, layer2: bool):
    """layer2: MP1 + b1 + ReLU + @(W2 Wp) -> T2. else: MP2 + bpp -> y (bf16)."""
    import concourse.bacc as bacc
    import concourse.mybir as mybir
    import concourse.tile as tile

    dt = mybir.dt
    F = cfg.HID if layer2 else cfg.OUT           # message feature width
    FCH = F // 128
    NT, TG = cfg.NTILES, cfg.TG
    tot = plan.tot
    mdt = dt.float8e3 if cfg.fp8_msg else dt.bfloat16
    full = layer2 and cfg.full_slab_l2
    GC = 64 if full else 128
    SLAB = plan.SLAB2 if full else plan.SLAB
    sdt = dt.float8e3 if full else dt.bfloat16
    nc = bacc.Bacc(None, target_bir_lowering=False)

    msg = nc.dram_tensor("msg", [128, tot * F], mdt, kind="ExternalInput")
    wsl = nc.dram_tensor("wsl", [128, SLAB], sdt, kind="ExternalInput")
    bias = nc.dram_tensor("bias", [128, F], dt.bfloat16, kind="ExternalInput")
    ident = nc.dram_tensor("ident", [128, 128], dt.bfloat16,
                           kind="ExternalInput")
    if layer2:
        wnext = nc.dram_tensor("wnext", [128, FCH * cfg.OUT], dt.bfloat16,
                               kind="ExternalInput")
    out = nc.dram_tensor("out", [128, NT * cfg.OUT], dt.bfloat16,
                         kind="ExternalOutput")

    # slab pieces: a small first piece (2 tiles) so the first matmul only
    # waits ~1 tile worth of columns, then TG-tile pieces
    cut_tiles = [0, min(1, NT)] + list(range(TG, NT, TG)) + [NT]
    cut_tiles = sorted(set(cut_tiles))
    cut_chunks = [int(plan.base[t]) for t in cut_tiles]
    if full:
        cut_cols = [c * 128 for c in cut_chunks]
    else:
        cut_cols = [int(plan.slab_off[c]) if c < tot else plan.SLAB
                    for c in cut_chunks]
    cut_cols[-1] = SLAB

    # message group boundaries: small first group, then GC-chunk groups
    gb = [0, min(4, tot)]
    while gb[-1] < tot:
        gb.append(min(gb[-1] + GC, tot))
    import numpy as _np
    chunk2grp = _np.searchsorted(_np.asarray(gb), _np.arange(tot),
                                 side="right") - 1

    with tile.TileContext(nc) as tc, ExitStack() as ctx:
        consts = ctx.enter_context(tc.tile_pool(name="consts", bufs=1))
        mg = ctx.enter_context(tc.tile_pool(name="mg", bufs=3))
        work = ctx.enter_context(tc.tile_pool(name="work", bufs=4))
        outs = ctx.enter_context(tc.tile_pool(name="outs", bufs=2))
        psmp = ctx.enter_context(tc.tile_pool(name="psmp", bufs=4, space="PSUM"))
        if layer2:
            pstr = ctx.enter_context(tc.tile_pool(name="pstr", bufs=2,
                                                  space="PSUM"))
            psmm = ctx.enter_context(tc.tile_pool(name="psmm", bufs=2,
                                                  space="PSUM"))

        bias_sb = consts.tile([128, F], dt.bfloat16, tag="bias")
        nc.scalar.dma_start(bias_sb[:], bias[:])
        ident_sb = consts.tile([128, 128], dt.bfloat16, tag="ident")
        nc.scalar.dma_start(ident_sb[:], ident[:])
        wsl_sb = consts.tile([128, SLAB], sdt, tag="wsl")
        nc.scalar.dma_start(wsl_sb[:, cut_cols[0]:cut_cols[1]],
                            wsl[:, cut_cols[0]:cut_cols[1]])
        if layer2:
            wnext_sb = consts.tile([128, FCH * cfg.OUT], dt.bfloat16,
                                   tag="wnext")
            nc.scalar.dma_start(wnext_sb[:], wnext[:])
        for i in range(1, len(cut_cols) - 1):
            if cut_cols[i + 1] > cut_cols[i]:
                nc.scalar.dma_start(wsl_sb[:, cut_cols[i]:cut_cols[i + 1]],
                                    wsl[:, cut_cols[i]:cut_cols[i + 1]])

        gtiles = {}

        def group_tile(g):
            if g in gtiles:
                return gtiles[g]
            ck = gb[g + 1] - gb[g]
            t = mg.tile([128, GC * F], mdt)
            nc.sync.dma_start(t[:, : ck * F],
                              msg[:, gb[g] * F: gb[g + 1] * F])
            gtiles[g] = t
            return t

        o_g = None

        def tile_chunks(t):
            return [c for c in range(int(plan.base[t]), int(plan.base[t + 1]))
                    if int(plan.cM[c]) > 0]

        def chunk_mm(ps, c, last):
            if full:
                M, B, off = 128, 0, c * 128
            else:
                M = int(plan.cM[c])
                B = int(plan.cB[c])
                off = int(plan.slab_off[c])
            g = int(chunk2grp[c])
            gt = group_tile(g)
            slot = c - gb[g]
            nc.tensor.matmul(
                ps[B:B + M, :],
                wsl_sb[:, off:off + M],
                gt[:, slot * F: (slot + 1) * F],
                start=False, stop=last,
                skip_group_check=True,
            )

        def out_write(t):
            # o_g slice for tile t was filled; flush the group at boundaries
            if t % TG == TG - 1 or t == NT - 1:
                g0 = (t // TG) * TG
                nt = t - g0 + 1
                nc.scalar.dma_start(
                    out[:, g0 * cfg.OUT: (g0 + nt) * cfg.OUT],
                    o_g[:, : nt * cfg.OUT])

        def oslice_for(t):
            nonlocal o_g
            if t % TG == 0:
                o_g = outs.tile([128, TG * cfg.OUT], dt.bfloat16)
            return o_g[:, (t % TG) * cfg.OUT: (t % TG + 1) * cfg.OUT]

        if layer2:
            # software-pipelined post-processing: each tile's PE post work
            # (transposes, wnext) is deferred 1-2 tiles so the scalar relu /
            # vector copy latencies hide behind the next tile's chunk stream
            # (the PE executes its queue in order; only LDWEIGHTS reorders)
            acts = {}    # t -> act tile (awaiting transpose)
            actTs = {}   # t -> actT tile (awaiting wnext matmul)
            ps2s = {}    # t -> psum out (awaiting final copy)

            def stage1(t, ps):          # scalar: relu out of PSUM
                act = work.tile([128, F], dt.bfloat16)
                nc.scalar.activation(act[:], ps[:],
                                     mybir.ActivationFunctionType.Relu)
                acts[t] = act

            def stage2(t):              # PE: transpose + vector copy
                act = acts.pop(t)
                trp = pstr.tile([128, F], dt.bfloat16)
                for c in range(FCH):
                    nc.tensor.transpose(trp[:, c * 128:(c + 1) * 128],
                                        act[:, c * 128:(c + 1) * 128],
                                        ident_sb[:])
                actT = work.tile([128, F], dt.bfloat16)
                nc.vector.tensor_copy(actT[:], trp[:])
                actTs[t] = actT

            def stage3(t):              # PE: @ (W2 Wp)
                actT = actTs.pop(t)
                ps2 = psmm.tile([128, cfg.OUT], dt.float32)
                for c in range(FCH):
                    nc.tensor.matmul(ps2[:], actT[:, c * 128:(c + 1) * 128],
                                     wnext_sb[:, c * cfg.OUT:(c + 1) * cfg.OUT],
                                     start=(c == 0), stop=(c == FCH - 1))
                ps2s[t] = ps2

            def stage4(t):              # scalar: copy out + flush
                ps2 = ps2s.pop(t)
                nc.scalar.activation(oslice_for(t), ps2[:],
                                     mybir.ActivationFunctionType.Copy)
                out_write(t)

            for t in range(NT + 2):
                if t < NT:
                    chunks = tile_chunks(t)
                    ps = psmp.tile([128, F], dt.float32)
                    nc.tensor.matmul(ps[:], ident_sb[:], bias_sb[:],
                                     start=True, stop=False,
                                     skip_group_check=True)
                    for j, c in enumerate(chunks):
                        chunk_mm(ps, c, j == len(chunks) - 1)
                    stage1(t, ps)
                if t - 1 >= 0 and t - 1 < NT:
                    stage2(t - 1)
                if t - 2 >= 0:
                    stage3(t - 2)
                    stage4(t - 2)
        else:
            for t in range(NT):
                chunks = tile_chunks(t)
                ps = psmp.tile([128, F], dt.float32)
                nc.tensor.matmul(ps[:], ident_sb[:], bias_sb[:],
                                 start=True, stop=False, skip_group_check=True)
                for j, c in enumerate(chunks):
                    chunk_mm(ps, c, j == len(chunks) - 1)
                oslice = oslice_for(t)
                if t % 2 == 0:
                    nc.scalar.activation(oslice, ps[:],
                                         mybir.ActivationFunctionType.Copy)
                else:
                    nc.vector.tensor_copy(oslice, ps[:])
                out_write(t)

    nc.finalize()
    return nc


# ---------------------------------------------------------------- host packing

def _quant_rows(table):
    """fp8 e3m4 per-row quantization. Returns (q [R,F] fp8, scale [R] f32)."""
    a = np.asarray(table, np.float32)
    s = np.abs(a).max(axis=1) / FP8_MAX
    s[s == 0] = 1.0
    q = (a / s[:, None]).astype(FP8)
    return q, s.astype(np.float32)


def _pack_l1_inputs(cfg: Cfg, plan: Plan, x, W1):
    KCH = cfg.IN_DIM // 128
    G1, W = cfg.G1, cfg.NG1 * 128
    w1r = np.zeros((128, KCH * cfg.HID), BF16)
    for c in range(KCH):
        w1r[:, c * cfg.HID:(c + 1) * cfg.HID] = \
            W1[c * 128:(c + 1) * 128, :].astype(BF16)
    xdt = FP8 if cfg.fp8_x else BF16
    maps = []
    for k in range(cfg.NCORES):
        xs = np.zeros((G1 * W, cfg.IN_DIM), np.float32)
        xs[:cfg.ND] = x[plan.nodes[k]]
        if cfg.fp8_x:
            # global scale; its inverse is folded into this core's W1 copy
            m = max(float(np.abs(xs).max()), 1e-30)
            xs = xs * (FP8_MAX / m)
        # [g, n, c, kk] -> [kk, g, c, n]
        xtr = np.ascontiguousarray(
            xs.reshape(G1, W, KCH, 128).transpose(3, 0, 2, 1)
        ).reshape(128, G1 * KCH * W).astype(xdt)
        if cfg.fp8_x:
            mp = {"xt": xtr, "w1": (w1r.astype(np.float32) * (m / FP8_MAX)
                                    ).astype(BF16)}
        else:
            mp = {"xt": xtr, "w1": w1r}
        maps.append(mp)
    return maps


def _unpack_h1t(cfg: Cfg, arr):
    # h1t [128, G1*OCH*W] -> H1 [NP, HID]
    OCH = cfg.HID // 128
    G1, W = cfg.G1, cfg.NG1 * 128
    a = np.asarray(arr).reshape(128, G1, OCH, W)
    # H1[g*W+n, o*128+p] = a[p, g, o, n]
    return np.ascontiguousarray(
        a.transpose(1, 3, 2, 0)).reshape(G1 * W, cfg.HID)[:cfg.NP]


def _untile_out(cfg: Cfg, arr):
    # [128, NT*OUT] -> [NP, OUT]
    return np.ascontiguousarray(
        np.asarray(arr).reshape(128, cfg.NTILES, cfg.OUT).transpose(1, 0, 2)
    ).reshape(cfg.NP, cfg.OUT)


def _pack_mp_inputs(cfg: Cfg, plan: Plan, table, Wn, b, layer2):
    F = cfg.HID if layer2 else cfg.OUT
    full = layer2 and cfg.full_slab_l2
    ident = np.eye(128, dtype=BF16)
    if cfg.fp8_msg:
        qtab, scale = _quant_rows(table)
    else:
        qtab, scale = np.asarray(table).astype(BF16), np.ones(
            table.shape[0], np.float32)
    maps = []
    for k in range(cfg.NCORES):
        gathered = qtab[plan.midx[k]]                    # [tot*128, F]
        msg = np.ascontiguousarray(
            gathered.reshape(plan.tot, 128, F).transpose(1, 0, 2)
        ).reshape(128, plan.tot * F)
        if full:
            # fp8 slab scaled by 1/alpha; fold alpha into bias and wnext:
            # relu((MP + b1)/a) @ (a*W2p) == relu(MP + b1) @ W2p  (a > 0)
            wslk, alpha = plan.build_slab_full(k, scale)
            biask = np.tile((b / alpha).astype(BF16)[None, :], (128, 1))
            wnk = Wn * alpha
        else:
            wslk = plan.build_slab(k, scale)
            biask = np.tile(b.astype(BF16)[None, :], (128, 1))
            wnk = Wn
        m = {
            "msg": msg,
            "wsl": wslk,
            "bias": biask,
            "ident": ident,
        }
        if layer2:
            FCH = cfg.HID // 128
            wnr = np.zeros((128, FCH * cfg.OUT), BF16)
            for c in range(FCH):
                wnr[:, c * cfg.OUT:(c + 1) * cfg.OUT] = \
                    wnk[c * 128:(c + 1) * 128, :].astype(BF16)
            m["wnext"] = wnr
        maps.append(m)
    return maps


# ---------------------------------------------------------------- driver

def _run(nc, in_maps, cfg, trace=False):
    from concourse.bass_utils import run_bass_kernel_spmd
    res = run_bass_kernel_spmd(nc, in_maps, list(range(cfg.NCORES)), trace=trace)
    return res


def kernel_run(inputs, cfg=None, trace=False, sim=False):
    cfg = cfg or Cfg()
    x = np.asarray(inputs["x"], np.float32)
    plan = Plan(cfg, np.asarray(inputs["edge_index"]),
                np.asarray(inputs["edge_weight"], np.float32))
    W1 = np.asarray(inputs["W1"], np.float32)
    b1 = np.asarray(inputs["b1"], np.float32)
    W2 = np.asarray(inputs["W2"], np.float32)
    b2 = np.asarray(inputs["b2"], np.float32)
    Wp = np.asarray(inputs["Wp"], np.float32)
    bp = np.asarray(inputs["bp"], np.float32)

    results = []

    def run(build, maps, outname):
        nc = build()
        if sim:
            from concourse.bass_interp import CoreSim
            outs = []
            for k in range(cfg.NCORES):
                s = CoreSim(nc)
                for name, arr in maps[k].items():
                    s.tensor(name)[:] = arr
                s.simulate()
                outs.append({outname: s.tensor(outname).copy()})
            results.append(None)
            return outs
        r = _run(nc, maps, cfg, trace=trace)
        results.append(r)
        return r.results

    # fold the post-projection into layer 2: A(relu1@W2)@Wp = A(relu1@(W2@Wp))
    W2p = (W2 @ Wp).astype(np.float32)
    bpp = (b2 @ Wp + bp).astype(np.float32)

    def asnp(a, dtype):
        a = np.asarray(a)
        return a if a.dtype == dtype else a.view(dtype)

    r1 = run(lambda: _build_l1(cfg), _pack_l1_inputs(cfg, plan, x, W1), "h1t")
    T1 = np.concatenate(
        [_unpack_h1t(cfg, asnp(r["h1t"], BF16)) for r in r1], axis=0)

    r2 = run(lambda: _build_mp(cfg, plan, True),
             _pack_mp_inputs(cfg, plan, T1, W2p, b1, True), "out")
    T2 = np.concatenate(
        [_untile_out(cfg, asnp(r["out"], BF16)) for r in r2], axis=0)

    r3 = run(lambda: _build_mp(cfg, plan, False),
             _pack_mp_inputs(cfg, plan, T2, None, bpp, False), "out")

    y = np.empty((cfg.N, cfg.OUT), np.float32)
    for k in range(cfg.NCORES):
        shard = _untile_out(cfg, asnp(r3[k]["out"], BF16)).astype(np.float32)
        y[plan.nodes[k]] = shard[:cfg.ND]
    return y, results


def kernel(**inputs):
    y, _ = kernel_run(inputs)
    return y


# revision 22
# speedup vs baseline: 1.0579x; 1.0579x over previous
"""Trainium2 Bass kernel: 2-layer GCN (GCNConv -> ReLU -> GCNConv -> Linear).

Strategy (8 NeuronCores, SPMD, 3 launches with host-side exchange):
  - Destination-node sharding with degree-sorted serpentine assignment.
  - The host reorders activation tables into *edge order* between launches
    (pure data movement / dtype casts), so each launch streams its operands
    sequentially at HWDGE line rate -- no on-device gather descriptors.
      L1: H1 = X @ W1    (transposed orientation: W1 stationary, node dim
                          streams in N=512 matmuls; emits H1^T, host detiles)
      L2: MP1 + bias + ReLU, @ (W2 Wp) (segment reduction via PE one-hot
                                        weight-slab matmuls over pre-ordered
                                        fp8 message chunks)
      L3: MP2 + bias                   (same geometry, F=128)
  - Segment reduction: edges (incl. self loops) sorted by destination; each
    chunk of 128 edge slots is one [128, F] message tile; a [128, M] slab
    block (lhsT, norm weights scattered at (slot, dst-lane)) contracts it
    into the destination rows of a PSUM tile.  Bias via a leading
    identity-x-bias matmul (start covers the full region).
  - Messages are fp8 e3m4 with per-table-row scales; the scale of each
    edge's source row is folded into that edge's slab weight (bf16).
"""

from contextlib import ExitStack
from dataclasses import dataclass, field

import numpy as np
import ml_dtypes

BF16 = ml_dtypes.bfloat16
FP8 = ml_dtypes.float8_e3m4
FP8_MAX = 14.0
FP32 = np.float32


# ---------------------------------------------------------------- config

@dataclass
class Cfg:
    N: int = 50000
    IN_DIM: int = 512
    HID: int = 256
    OUT: int = 128
    NCORES: int = 8
    GC: int = 64          # message chunks per DMA group
    TG: int = 8           # dest tiles per output DMA group
    NG1: int = 4          # L1: tiles per node group (N=512 streams)
    fp8_msg: bool = True
    fp8_x: bool = True
    full_slab_l2: bool = True   # L2: M=128 windows + fp8 slab -> FWL hides LDW

    ND: int = field(init=False)
    NTILES: int = field(init=False)
    NP: int = field(init=False)
    TROWS: int = field(init=False)
    G1: int = field(init=False)

    def __post_init__(self):
        self.ND = self.N // self.NCORES
        self.NTILES = (self.ND + 127) // 128
        self.NP = self.NTILES * 128
        self.TROWS = self.NCORES * self.NP
        self.G1 = -(-self.NTILES // self.NG1)


# ---------------------------------------------------------------- planner

class Plan:
    """Static (cross-core identical) geometry + per-core data arrays."""

    def __init__(self, cfg: Cfg, edge_index, edge_weight):
        self.cfg = cfg
        N, ND, NP, NT = cfg.N, cfg.ND, cfg.NP, cfg.NTILES
        NC = cfg.NCORES

        # --- gcn_norm with self loops (host: O(E) index/weight preprocessing)
        row = np.asarray(edge_index[0], np.int64)
        col = np.asarray(edge_index[1], np.int64)
        w = np.asarray(edge_weight, np.float64)
        deg = np.ones(N, np.float64)          # self-loop weight 1.0
        np.add.at(deg, col, w)
        dinv = np.where(deg > 0, 1.0 / np.sqrt(deg), 0.0)
        nrm = (dinv[row] * w * dinv[col]).astype(np.float32)

        # --- global degree-sorted serpentine node->(core, lane) assignment
        degi = np.bincount(col, minlength=N)
        ranks = np.argsort(-degi, kind="stable")    # rank r -> node
        r = np.arange(N)
        blk = r // NC
        corepos = np.where(blk % 2 == 0, r % NC, NC - 1 - (r % NC))
        lane_r = blk
        lane_global = np.empty(N, np.int64)        # node -> core*NP + lane
        lane_global[ranks] = corepos * NP + lane_r
        self.nodes = []                             # per core: lane -> node id
        for k in range(NC):
            nk = np.empty(ND, np.int64)
            sel = corepos == k
            nk[lane_r[sel]] = ranks[sel]
            self.nodes.append(nk)

        # --- edge stream incl. self loops, sorted by destination
        row_all = np.concatenate([row, np.arange(N, dtype=np.int64)])
        col_all = np.concatenate([col, np.arange(N, dtype=np.int64)])
        w_all = np.concatenate([nrm, (dinv * dinv).astype(np.float32)])

        src_t = lane_global[row_all]                # table row of the source
        dstg = lane_global[col_all]
        dst_core = dstg // NP
        dlane = dstg % NP
        dtile = dlane // 128
        dl = dlane - dtile * 128

        order = np.lexsort((dl, dtile, dst_core))
        sc = dst_core[order]
        st = dtile[order]
        sl = dl[order]
        ssrc = src_t[order]
        sw = w_all[order]

        key = sc * NT + st
        cnt = np.bincount(key, minlength=NC * NT).reshape(NC, NT)
        CH = (-(-cnt // 128)).max(axis=0)            # [NT] static chunks/tile
        self.base = np.concatenate([[0], np.cumsum(CH)]).astype(np.int64)
        self.tot = int(self.base[-1])

        seg_start = np.concatenate(
            [[0], np.cumsum(np.bincount(key, minlength=NC * NT))])[:-1]
        rank = np.arange(len(key)) - seg_start[key]
        chunk = self.base[st] + rank // 128          # static chunk id
        lanepos = rank % 128

        # --- cross-core chunk windows with legal matmul out bases (0/32/64)
        mn = np.full(self.tot, 128, np.int64)
        mx = np.full(self.tot, -1, np.int64)
        np.minimum.at(mn, chunk, sl)
        np.maximum.at(mx, chunk, sl)
        empty = mx < 0
        mn[empty] = 0
        mx[empty] = -1
        b32 = (mn // 32) * 32
        m32 = mx - b32 + 1
        b64 = (mn // 64) * 64
        m64 = mx - b64 + 1
        ok32 = (m32 <= 32) & (b32 <= 64)
        ok64 = m64 <= 64
        B = np.where(ok32, b32, np.where(ok64, b64, 0))
        M = np.where(ok32, m32, np.where(ok64, m64, mx + 1))
        M[empty] = 0
        B[empty] = 0
        self.cB = B
        self.cM = M
        self.slab_off = np.concatenate([[0], np.cumsum(M)])[:-1]
        self.SLAB = max(int(M.sum()), 1)

        # full-window layout (M=128 for every chunk): weights have 128
        # columns so the compiler enables FWL and LDWEIGHTS hides behind the
        # matmul stream
        self.SLAB2 = self.tot * 128

        # --- per-core arrays (slab values are built per launch: the fp8
        # per-row scale of each edge's source folds into its weight)
        self.midx = []    # slot -> table row, len tot*128
        self.edata = []   # (lanepos, slabcol, fullcol, weight f32, src row)
        for k in range(NC):
            m = sc == k
            idx = np.zeros(self.tot * 128, np.int64)
            idx[chunk[m] * 128 + lanepos[m]] = ssrc[m]
            self.midx.append(idx)
            self.edata.append((lanepos[m],
                               self.slab_off[chunk[m]] + sl[m] - B[chunk[m]],
                               chunk[m] * 128 + sl[m],
                               sw[m].astype(np.float32),
                               ssrc[m]))

    def build_slab(self, k, row_scale):
        lp, col, _, w, src = self.edata[k]
        slab = np.zeros((128, self.SLAB), np.float32)
        slab[lp, col] = w * row_scale[src]
        return slab.astype(BF16)

    def build_slab_full(self, k, row_scale):
        """fp8 full-window slab + the inverse of its global scale (alpha)."""
        lp, _, fcol, w, src = self.edata[k]
        vals = w * row_scale[src]
        alpha = max(float(np.abs(vals).max()), 1e-30) / FP8_MAX
        slab = np.zeros((128, self.SLAB2), np.float32)
        slab[lp, fcol] = vals / alpha
        return slab.astype(FP8), alpha


# ---------------------------------------------------------------- bass builders

def _build_l1(cfg: Cfg):
    import concourse.bacc as bacc
    import concourse.mybir as mybir
    import concourse.tile as tile

    dt = mybir.dt
    nc = bacc.Bacc(None, target_bir_lowering=False)
    KCH = cfg.IN_DIM // 128          # 4 contraction chunks
    OCH = cfg.HID // 128             # 2 output halves
    G1, NG1 = cfg.G1, cfg.NG1
    W = NG1 * 128                    # nodes per group (512)
    xdt = dt.float8e3 if cfg.fp8_x else dt.bfloat16
    xt = nc.dram_tensor("xt", [128, G1 * KCH * W], xdt, kind="ExternalInput")
    w1 = nc.dram_tensor("w1", [128, KCH * cfg.HID], dt.bfloat16,
                        kind="ExternalInput")
    # h1t[p, (g*OCH + o)*W + n] = H1[g*W + n, o*128 + p]
    h1t = nc.dram_tensor("h1t", [128, G1 * OCH * W], dt.bfloat16,
                         kind="ExternalOutput")

    with tile.TileContext(nc) as tc, ExitStack() as ctx:
        consts = ctx.enter_context(tc.tile_pool(name="consts", bufs=1))
        outs = ctx.enter_context(tc.tile_pool(name="outs", bufs=2))
        psum = ctx.enter_context(tc.tile_pool(name="psum", bufs=4, space="PSUM"))

        w1_sb = consts.tile([128, KCH * cfg.HID], dt.bfloat16, tag="w1")
        nc.sync.dma_start(w1_sb[:], w1[:])

        # preload every x group up front: the PE then streams without gaps
        # (staying busy keeps the HAM clock-gate at full rate)
        xgs = []
        for g in range(G1):
            xg_t = consts.tile([128, KCH * W], xdt, tag=f"xg{g}")
            nc.sync.dma_start(xg_t[:], xt[:, g * KCH * W: (g + 1) * KCH * W])
            xgs.append(xg_t)

        for g in range(G1):
            xg_t = xgs[g]
            o_g = outs.tile([128, OCH * W], dt.bfloat16)
            for o in range(OCH):
                ps = psum.tile([128, W], dt.float32)
                for c in range(KCH):
                    # lhsT = W1 chunk [128k, 128feat]; rhs = x^T [128k, W]
                    nc.tensor.matmul(
                        ps[:],
                        w1_sb[:, c * cfg.HID + o * 128: c * cfg.HID + (o + 1) * 128],
                        xg_t[:, c * W: (c + 1) * W],
                        start=(c == 0), stop=(c == KCH - 1),
                    )
                if o % 2 == 0:
                    nc.scalar.activation(o_g[:, o * W: (o + 1) * W], ps[:],
                                         mybir.ActivationFunctionType.Copy)
                else:
                    nc.vector.tensor_copy(o_g[:, o * W: (o + 1) * W], ps[:])
            nc.scalar.dma_start(h1t[:, g * OCH * W: (g + 1) * OCH * W], o_g[:])
    nc.finalize()
    return nc


def _build_mp(cfg: Cfg, plan: Plan, layer2: bool):
    """layer2: MP1 + b1 + ReLU + @(W2 Wp) -> T2. else: MP2 + bpp -> y (bf16)."""
    import concourse.bacc as bacc
    import concourse.mybir as mybir
    import concourse.tile as tile

    dt = mybir.dt
    F = cfg.HID if layer2 else cfg.OUT           # message feature width
    FCH = F // 128
    NT, TG = cfg.NTILES, cfg.TG
    tot = plan.tot
    mdt = dt.float8e3 if cfg.fp8_msg else dt.bfloat16
    full = layer2 and cfg.full_slab_l2
    GC = 32 if full else cfg.GC
    SLAB = plan.SLAB2 if full else plan.SLAB
    sdt = dt.float8e3 if full else dt.bfloat16
    nc = bacc.Bacc(None, target_bir_lowering=False)

    msg = nc.dram_tensor("msg", [128, tot * F], mdt, kind="ExternalInput")
    wsl = nc.dram_tensor("wsl", [128, SLAB], sdt, kind="ExternalInput")
    bias = nc.dram_tensor("bias", [128, F], dt.bfloat16, kind="ExternalInput")
    ident = nc.dram_tensor("ident", [128, 128], dt.bfloat16,
                           kind="ExternalInput")
    if layer2:
        wnext = nc.dram_tensor("wnext", [128, FCH * cfg.OUT], dt.bfloat16,
                               kind="ExternalInput")
    out = nc.dram_tensor("out", [128, NT * cfg.OUT], dt.bfloat16,
                         kind="ExternalOutput")

    # slab pieces: a small first piece (2 tiles) so the first matmul only
    # waits ~1 tile worth of columns, then TG-tile pieces
    cut_tiles = [0, min(2, NT)] + list(range(TG, NT, TG)) + [NT]
    cut_tiles = sorted(set(cut_tiles))
    cut_chunks = [int(plan.base[t]) for t in cut_tiles]
    if full:
        cut_cols = [c * 128 for c in cut_chunks]
    else:
        cut_cols = [int(plan.slab_off[c]) if c < tot else plan.SLAB
                    for c in cut_chunks]
    cut_cols[-1] = SLAB

    # message group boundaries: small first group, then GC-chunk groups
    gb = [0, min(8, tot)]
    while gb[-1] < tot:
        gb.append(min(gb[-1] + GC, tot))
    import numpy as _np
    chunk2grp = _np.searchsorted(_np.asarray(gb), _np.arange(tot),
                                 side="right") - 1

    with tile.TileContext(nc) as tc, ExitStack() as ctx:
        consts = ctx.enter_context(tc.tile_pool(name="consts", bufs=1))
        mg = ctx.enter_context(tc.tile_pool(name="mg", bufs=4))
        work = ctx.enter_context(tc.tile_pool(name="work", bufs=4))
        outs = ctx.enter_context(tc.tile_pool(name="outs", bufs=2))
        psmp = ctx.enter_context(tc.tile_pool(name="psmp", bufs=4, space="PSUM"))
        if layer2:
            pstr = ctx.enter_context(tc.tile_pool(name="pstr", bufs=2,
                                                  space="PSUM"))
            psmm = ctx.enter_context(tc.tile_pool(name="psmm", bufs=2,
                                                  space="PSUM"))

        bias_sb = consts.tile([128, F], dt.bfloat16, tag="bias")
        nc.scalar.dma_start(bias_sb[:], bias[:])
        ident_sb = consts.tile([128, 128], dt.bfloat16, tag="ident")
        nc.scalar.dma_start(ident_sb[:], ident[:])
        wsl_sb = consts.tile([128, SLAB], sdt, tag="wsl")
        nc.scalar.dma_start(wsl_sb[:, cut_cols[0]:cut_cols[1]],
                            wsl[:, cut_cols[0]:cut_cols[1]])
        if layer2:
            wnext_sb = consts.tile([128, FCH * cfg.OUT], dt.bfloat16,
                                   tag="wnext")
            nc.scalar.dma_start(wnext_sb[:], wnext[:])
        for i in range(1, len(cut_cols) - 1):
            if cut_cols[i + 1] > cut_cols[i]:
                nc.scalar.dma_start(wsl_sb[:, cut_cols[i]:cut_cols[i + 1]],
                                    wsl[:, cut_cols[i]:cut_cols[i + 1]])

        gtiles = {}

        def group_tile(g):
            if g in gtiles:
                return gtiles[g]
            ck = gb[g + 1] - gb[g]
            t = mg.tile([128, GC * F], mdt)
            nc.sync.dma_start(t[:, : ck * F],
                              msg[:, gb[g] * F: gb[g + 1] * F])
            gtiles[g] = t
            return t

        o_g = None

        def tile_chunks(t):
            return [c for c in range(int(plan.base[t]), int(plan.base[t + 1]))
                    if int(plan.cM[c]) > 0]

        def chunk_mm(ps, c, last):
            if full:
                M, B, off = 128, 0, c * 128
            else:
                M = int(plan.cM[c])
                B = int(plan.cB[c])
                off = int(plan.slab_off[c])
            g = int(chunk2grp[c])
            gt = group_tile(g)
            slot = c - gb[g]
            nc.tensor.matmul(
                ps[B:B + M, :],
                wsl_sb[:, off:off + M],
                gt[:, slot * F: (slot + 1) * F],
                start=False, stop=last,
                skip_group_check=True,
            )

        def out_write(t):
            # o_g slice for tile t was filled; flush the group at boundaries
            if t % TG == TG - 1 or t == NT - 1:
                g0 = (t // TG) * TG
                nt = t - g0 + 1
                nc.scalar.dma_start(
                    out[:, g0 * cfg.OUT: (g0 + nt) * cfg.OUT],
                    o_g[:, : nt * cfg.OUT])

        def oslice_for(t):
            nonlocal o_g
            if t % TG == 0:
                o_g = outs.tile([128, TG * cfg.OUT], dt.bfloat16)
            return o_g[:, (t % TG) * cfg.OUT: (t % TG + 1) * cfg.OUT]

        if layer2:
            # software-pipelined post-processing: each tile's PE post work
            # (transposes, wnext) is deferred 1-2 tiles so the scalar relu /
            # vector copy latencies hide behind the next tile's chunk stream
            # (the PE executes its queue in order; only LDWEIGHTS reorders)
            acts = {}    # t -> act tile (awaiting transpose)
            actTs = {}   # t -> actT tile (awaiting wnext matmul)
            ps2s = {}    # t -> psum out (awaiting final copy)

            def stage1(t, ps):          # scalar: relu out of PSUM
                act = work.tile([128, F], dt.bfloat16)
                nc.scalar.activation(act[:], ps[:],
                                     mybir.ActivationFunctionType.Relu)
                acts[t] = act

            def stage2(t):              # PE: transpose + vector copy
                act = acts.pop(t)
                trp = pstr.tile([128, F], dt.bfloat16)
                for c in range(FCH):
                    nc.tensor.transpose(trp[:, c * 128:(c + 1) * 128],
                                        act[:, c * 128:(c + 1) * 128],
                                        ident_sb[:])
                actT = work.tile([128, F], dt.bfloat16)
                nc.vector.tensor_copy(actT[:], trp[:])
                actTs[t] = actT

            def stage3(t):              # PE: @ (W2 Wp)
                actT = actTs.pop(t)
                ps2 = psmm.tile([128, cfg.OUT], dt.float32)
                for c in range(FCH):
                    nc.tensor.matmul(ps2[:], actT[:, c * 128:(c + 1) * 128],
                                     wnext_sb[:, c * cfg.OUT:(c + 1) * cfg.OUT],
                                     start=(c == 0), stop=(c == FCH - 1))
                ps2s[t] = ps2

            def stage4(t):              # scalar: copy out + flush
                ps2 = ps2s.pop(t)
                nc.scalar.activation(oslice_for(t), ps2[:],
                                     mybir.ActivationFunctionType.Copy)
                out_write(t)

            for t in range(NT + 2):
                if t < NT:
                    chunks = tile_chunks(t)
                    ps = psmp.tile([128, F], dt.float32)
                    nc.tensor.matmul(ps[:], ident_sb[:], bias_sb[:],
                                     start=True, stop=False,
                                     skip_group_check=True)
                    for j, c in enumerate(chunks):
                        chunk_mm(ps, c, j == len(chunks) - 1)
                    stage1(t, ps)
                if t - 1 >= 0 and t - 1 < NT:
                    stage2(t - 1)
                if t - 2 >= 0:
                    stage3(t - 2)
                    stage4(t - 2)
        else:
            for t in range(NT):
                chunks = tile_chunks(t)
                ps = psmp.tile([128, F], dt.float32)
                nc.tensor.matmul(ps[:], ident_sb[:], bias_sb[:],
                                 start=True, stop=False, skip_group_check=True)
                for j, c in enumerate(chunks):
                    chunk_mm(ps, c, j == len(chunks) - 1)
                oslice = oslice_for(t)
                if t % 2 == 0:
                    nc.scalar.activation(oslice, ps[:],
                                         mybir.ActivationFunctionType.Copy)
                else:
                    nc.vector.tensor_copy(oslice, ps[:])
                out_write(t)

    nc.finalize()
    return nc


# ---------------------------------------------------------------- host packing

def _quant_rows(table):
    """fp8 e3m4 per-row quantization. Returns (q [R,F] fp8, scale [R] f32)."""
    a = np.asarray(table, np.float32)
    s = np.abs(a).max(axis=1) / FP8_MAX
    s[s == 0] = 1.0
    q = (a / s[:, None]).astype(FP8)
    return q, s.astype(np.float32)


def _pack_l1_inputs(cfg: Cfg, plan: Plan, x, W1):
    KCH = cfg.IN_DIM // 128
    G1, W = cfg.G1, cfg.NG1 * 128
    w1r = np.zeros((128, KCH * cfg.HID), BF16)
    for c in range(KCH):
        w1r[:, c * cfg.HID:(c + 1) * cfg.HID] = \
            W1[c * 128:(c + 1) * 128, :].astype(BF16)
    xdt = FP8 if cfg.fp8_x else BF16
    maps = []
    for k in range(cfg.NCORES):
        xs = np.zeros((G1 * W, cfg.IN_DIM), np.float32)
        xs[:cfg.ND] = x[plan.nodes[k]]
        if cfg.fp8_x:
            # global scale; its inverse is folded into this core's W1 copy
            m = max(float(np.abs(xs).max()), 1e-30)
            xs = xs * (FP8_MAX / m)
        # [g, n, c, kk] -> [kk, g, c, n]
        xtr = np.ascontiguousarray(
            xs.reshape(G1, W, KCH, 128).transpose(3, 0, 2, 1)
        ).reshape(128, G1 * KCH * W).astype(xdt)
        if cfg.fp8_x:
            mp = {"xt": xtr, "w1": (w1r.astype(np.float32) * (m / FP8_MAX)
                                    ).astype(BF16)}
        else:
            mp = {"xt": xtr, "w1": w1r}
        maps.append(mp)
    return maps


def _unpack_h1t(cfg: Cfg, arr):
    # h1t [128, G1*OCH*W] -> H1 [NP, HID]
    OCH = cfg.HID // 128
    G1, W = cfg.G1, cfg.NG1 * 128
    a = np.asarray(arr).reshape(128, G1, OCH, W)
    # H1[g*W+n, o*128+p] = a[p, g, o, n]
    return np.ascontiguousarray(
        a.transpose(1, 3, 2, 0)).reshape(G1 * W, cfg.HID)[:cfg.NP]


def _untile_out(cfg: Cfg, arr):
    # [128, NT*OUT] -> [NP, OUT]
    return np.ascontiguousarray(
        np.asarray(arr).reshape(128, cfg.NTILES, cfg.OUT).transpose(1, 0, 2)
    ).reshape(cfg.NP, cfg.OUT)


def _pack_mp_inputs(cfg: Cfg, plan: Plan, table, Wn, b, layer2):
    F = cfg.HID if layer2 else cfg.OUT
    full = layer2 and cfg.full_slab_l2
    ident = np.eye(128, dtype=BF16)
    if cfg.fp8_msg:
        qtab, scale = _quant_rows(table)
    else:
        qtab, scale = np.asarray(table).astype(BF16), np.ones(
            table.shape[0], np.float32)
    maps = []
    for k in range(cfg.NCORES):
        gathered = qtab[plan.midx[k]]                    # [tot*128, F]
        msg = np.ascontiguousarray(
            gathered.reshape(plan.tot, 128, F).transpose(1, 0, 2)
        ).reshape(128, plan.tot * F)
        if full:
            # fp8 slab scaled by 1/alpha; fold alpha into bias and wnext:
            # relu((MP + b1)/a) @ (a*W2p) == relu(MP + b1) @ W2p  (a > 0)
            wslk, alpha = plan.build_slab_full(k, scale)
            biask = np.tile((b / alpha).astype(BF16)[None, :], (128, 1))
            wnk = Wn * alpha
        else:
            wslk = plan.build_slab(k, scale)
            biask = np.tile(b.astype(BF16)[None, :], (128, 1))
            wnk = Wn
        m = {
            "msg": msg,
            "wsl": wslk,
            "bias": biask,
            "ident": ident,
        }
        if layer2:
            FCH = cfg.HID // 128
            wnr = np.zeros((128, FCH * cfg.OUT), BF16)
            for c in range(FCH):
                wnr[:, c * cfg.OUT:(c + 1) * cfg.OUT] = \
                    wnk[c * 128:(c + 1) * 128, :].astype(BF16)
            m["wnext"] = wnr
        maps.append(m)
    return maps


# ---------------------------------------------------------------- driver

def _run(nc, in_maps, cfg, trace=False):
    from concourse.bass_utils import run_bass_kernel_spmd
    res = run_bass_kernel_spmd(nc, in_maps, list(range(cfg.NCORES)), trace=trace)
    return res


def kernel_run(inputs, cfg=None, trace=False, sim=False):
    cfg = cfg or Cfg()
    x = np.asarray(inputs["x"], np.float32)
    plan = Plan(cfg, np.asarray(inputs["edge_index"]),
                np.asarray(inputs["edge_weight"], np.float32))
    W1 = np.asarray(inputs["W1"], np.float32)
    b1 = np.asarray(inputs["b1"], np.float32)
    W2 = np.asarray(inputs["W2"], np.float32)
    b2 = np.asarray(inputs["b2"], np.float32)
    Wp = np.asarray(inputs["Wp"], np.float32)
    bp = np.asarray(inputs["bp"], np.float32)

    results = []

    def run(build, maps, outname):
        nc = build()
        if sim:
            from concourse.bass_interp import CoreSim
            outs = []
            for k in range(cfg.NCORES):
                s = CoreSim(nc)
                for name, arr in maps[k].items():
                    s.tensor(name)[:] = arr
                s.simulate()
                outs.append({outname: s.tensor(outname).copy()})
            results.append(None)
            return outs
        r = _run(nc, maps, cfg, trace=trace)
        results.append(r)
        return r.results

    # fold the post-projection into layer 2: A(relu1@W2)@Wp = A(relu1@(W2@Wp))
    W2p = (W2 @ Wp).astype(np.float32)
    bpp = (b2 @ Wp + bp).astype(np.float32)

    def asnp(a, dtype):
        a = np.asarray(a)
        return a if a.dtype == dtype else a.view(dtype)

    r1 = run(lambda: _build_l1(cfg), _pack_l1_inputs(cfg, plan, x, W1), "h1t")
    T1 = np.concatenate(
        [_unpack_h1t(cfg, asnp(r["h1t"], BF16)) for r in r1], axis=0)

    r2 = run(lambda: _build_mp(cfg, plan, True),
             _pack_mp_inputs(cfg, plan, T1, W2p, b1, True), "out")
    T2 = np.concatenate(
        [_untile_out(cfg, asnp(r["out"], BF16)) for r in r2], axis=0)

    r3 = run(lambda: _build_mp(cfg, plan, False),
             _pack_mp_inputs(cfg, plan, T2, None, bpp, False), "out")

    y = np.empty((cfg.N, cfg.OUT), np.float32)
    for k in range(cfg.NCORES):
        shard = _untile_out(cfg, asnp(r3[k]["out"], BF16)).astype(np.float32)
        y[plan.nodes[k]] = shard[:cfg.ND]
    return y, results


def kernel(**inputs):
    y, _ = kernel_run(inputs)
    return y


# revision 23
# speedup vs baseline: 1.0604x; 1.0024x over previous
"""Trainium2 Bass kernel: 2-layer GCN (GCNConv -> ReLU -> GCNConv -> Linear).

Strategy (8 NeuronCores, SPMD, 3 launches with host-side exchange):
  - Destination-node sharding with degree-sorted serpentine assignment.
  - The host reorders activation tables into *edge order* between launches
    (pure data movement / dtype casts), so each launch streams its operands
    sequentially at HWDGE line rate -- no on-device gather descriptors.
      L1: H1 = X @ W1    (transposed orientation: W1 stationary, node dim
                          streams in N=512 matmuls; emits H1^T, host detiles)
      L2: MP1 + bias + ReLU, @ (W2 Wp) (segment reduction via PE one-hot
                                        weight-slab matmuls over pre-ordered
                                        fp8 message chunks)
      L3: MP2 + bias                   (same geometry, F=128)
  - Segment reduction: edges (incl. self loops) sorted by destination; each
    chunk of 128 edge slots is one [128, F] message tile; a [128, M] slab
    block (lhsT, norm weights scattered at (slot, dst-lane)) contracts it
    into the destination rows of a PSUM tile.  Bias via a leading
    identity-x-bias matmul (start covers the full region).
  - Messages are fp8 e3m4 with per-table-row scales; the scale of each
    edge's source row is folded into that edge's slab weight (bf16).
"""

from contextlib import ExitStack
from dataclasses import dataclass, field

import numpy as np
import ml_dtypes

BF16 = ml_dtypes.bfloat16
FP8 = ml_dtypes.float8_e3m4
FP8_MAX = 14.0
FP32 = np.float32


# ---------------------------------------------------------------- config

@dataclass
class Cfg:
    N: int = 50000
    IN_DIM: int = 512
    HID: int = 256
    OUT: int = 128
    NCORES: int = 8
    GC: int = 64          # message chunks per DMA group
    TG: int = 8           # dest tiles per output DMA group
    NG1: int = 4          # L1: tiles per node group (N=512 streams)
    fp8_msg: bool = True
    fp8_x: bool = True
    full_slab_l2: bool = True   # L2: M=128 windows + fp8 slab -> FWL hides LDW

    ND: int = field(init=False)
    NTILES: int = field(init=False)
    NP: int = field(init=False)
    TROWS: int = field(init=False)
    G1: int = field(init=False)

    def __post_init__(self):
        self.ND = self.N // self.NCORES
        self.NTILES = (self.ND + 127) // 128
        self.NP = self.NTILES * 128
        self.TROWS = self.NCORES * self.NP
        self.G1 = -(-self.NTILES // self.NG1)


# ---------------------------------------------------------------- planner

class Plan:
    """Static (cross-core identical) geometry + per-core data arrays."""

    def __init__(self, cfg: Cfg, edge_index, edge_weight):
        self.cfg = cfg
        N, ND, NP, NT = cfg.N, cfg.ND, cfg.NP, cfg.NTILES
        NC = cfg.NCORES

        # --- gcn_norm with self loops (host: O(E) index/weight preprocessing)
        row = np.asarray(edge_index[0], np.int64)
        col = np.asarray(edge_index[1], np.int64)
        w = np.asarray(edge_weight, np.float64)
        deg = np.ones(N, np.float64)          # self-loop weight 1.0
        np.add.at(deg, col, w)
        dinv = np.where(deg > 0, 1.0 / np.sqrt(deg), 0.0)
        nrm = (dinv[row] * w * dinv[col]).astype(np.float32)

        # --- global degree-sorted serpentine node->(core, lane) assignment
        degi = np.bincount(col, minlength=N)
        ranks = np.argsort(-degi, kind="stable")    # rank r -> node
        r = np.arange(N)
        blk = r // NC
        corepos = np.where(blk % 2 == 0, r % NC, NC - 1 - (r % NC))
        lane_r = blk
        lane_global = np.empty(N, np.int64)        # node -> core*NP + lane
        lane_global[ranks] = corepos * NP + lane_r
        self.nodes = []                             # per core: lane -> node id
        for k in range(NC):
            nk = np.empty(ND, np.int64)
            sel = corepos == k
            nk[lane_r[sel]] = ranks[sel]
            self.nodes.append(nk)

        # --- edge stream incl. self loops, sorted by destination
        row_all = np.concatenate([row, np.arange(N, dtype=np.int64)])
        col_all = np.concatenate([col, np.arange(N, dtype=np.int64)])
        w_all = np.concatenate([nrm, (dinv * dinv).astype(np.float32)])

        src_t = lane_global[row_all]                # table row of the source
        dstg = lane_global[col_all]
        dst_core = dstg // NP
        dlane = dstg % NP
        dtile = dlane // 128
        dl = dlane - dtile * 128

        order = np.lexsort((dl, dtile, dst_core))
        sc = dst_core[order]
        st = dtile[order]
        sl = dl[order]
        ssrc = src_t[order]
        sw = w_all[order]

        key = sc * NT + st
        cnt = np.bincount(key, minlength=NC * NT).reshape(NC, NT)
        CH = (-(-cnt // 128)).max(axis=0)            # [NT] static chunks/tile
        self.base = np.concatenate([[0], np.cumsum(CH)]).astype(np.int64)
        self.tot = int(self.base[-1])

        seg_start = np.concatenate(
            [[0], np.cumsum(np.bincount(key, minlength=NC * NT))])[:-1]
        rank = np.arange(len(key)) - seg_start[key]
        chunk = self.base[st] + rank // 128          # static chunk id
        lanepos = rank % 128

        # --- cross-core chunk windows with legal matmul out bases (0/32/64)
        mn = np.full(self.tot, 128, np.int64)
        mx = np.full(self.tot, -1, np.int64)
        np.minimum.at(mn, chunk, sl)
        np.maximum.at(mx, chunk, sl)
        empty = mx < 0
        mn[empty] = 0
        mx[empty] = -1
        b32 = (mn // 32) * 32
        m32 = mx - b32 + 1
        b64 = (mn // 64) * 64
        m64 = mx - b64 + 1
        ok32 = (m32 <= 32) & (b32 <= 64)
        ok64 = m64 <= 64
        B = np.where(ok32, b32, np.where(ok64, b64, 0))
        M = np.where(ok32, m32, np.where(ok64, m64, mx + 1))
        M[empty] = 0
        B[empty] = 0
        self.cB = B
        self.cM = M
        self.slab_off = np.concatenate([[0], np.cumsum(M)])[:-1]
        self.SLAB = max(int(M.sum()), 1)

        # full-window layout (M=128 for every chunk): weights have 128
        # columns so the compiler enables FWL and LDWEIGHTS hides behind the
        # matmul stream
        self.SLAB2 = self.tot * 128

        # --- per-core arrays (slab values are built per launch: the fp8
        # per-row scale of each edge's source folds into its weight)
        self.midx = []    # slot -> table row, len tot*128
        self.edata = []   # (lanepos, slabcol, fullcol, weight f32, src row)
        for k in range(NC):
            m = sc == k
            idx = np.zeros(self.tot * 128, np.int64)
            idx[chunk[m] * 128 + lanepos[m]] = ssrc[m]
            self.midx.append(idx)
            self.edata.append((lanepos[m],
                               self.slab_off[chunk[m]] + sl[m] - B[chunk[m]],
                               chunk[m] * 128 + sl[m],
                               sw[m].astype(np.float32),
                               ssrc[m]))

    def build_slab(self, k, row_scale):
        lp, col, _, w, src = self.edata[k]
        slab = np.zeros((128, self.SLAB), np.float32)
        slab[lp, col] = w * row_scale[src]
        return slab.astype(BF16)

    def build_slab_full(self, k, row_scale):
        """fp8 full-window slab + the inverse of its global scale (alpha)."""
        lp, _, fcol, w, src = self.edata[k]
        vals = w * row_scale[src]
        alpha = max(float(np.abs(vals).max()), 1e-30) / FP8_MAX
        slab = np.zeros((128, self.SLAB2), np.float32)
        slab[lp, fcol] = vals / alpha
        return slab.astype(FP8), alpha


# ---------------------------------------------------------------- bass builders

def _build_l1(cfg: Cfg):
    import concourse.bacc as bacc
    import concourse.mybir as mybir
    import concourse.tile as tile

    dt = mybir.dt
    nc = bacc.Bacc(None, target_bir_lowering=False)
    KCH = cfg.IN_DIM // 128          # 4 contraction chunks
    OCH = cfg.HID // 128             # 2 output halves
    G1, NG1 = cfg.G1, cfg.NG1
    W = NG1 * 128                    # nodes per group (512)
    xdt = dt.float8e3 if cfg.fp8_x else dt.bfloat16
    xt = nc.dram_tensor("xt", [128, G1 * KCH * W], xdt, kind="ExternalInput")
    w1 = nc.dram_tensor("w1", [128, KCH * cfg.HID], dt.bfloat16,
                        kind="ExternalInput")
    # h1t[p, (g*OCH + o)*W + n] = H1[g*W + n, o*128 + p]
    h1t = nc.dram_tensor("h1t", [128, G1 * OCH * W], dt.bfloat16,
                         kind="ExternalOutput")

    with tile.TileContext(nc) as tc, ExitStack() as ctx:
        consts = ctx.enter_context(tc.tile_pool(name="consts", bufs=1))
        outs = ctx.enter_context(tc.tile_pool(name="outs", bufs=2))
        psum = ctx.enter_context(tc.tile_pool(name="psum", bufs=4, space="PSUM"))

        w1_sb = consts.tile([128, KCH * cfg.HID], dt.bfloat16, tag="w1")
        nc.sync.dma_start(w1_sb[:], w1[:])

        # preload every x group up front: the PE then streams without gaps
        # (staying busy keeps the HAM clock-gate at full rate)
        xgs = []
        for g in range(G1):
            xg_t = consts.tile([128, KCH * W], xdt, tag=f"xg{g}")
            nc.sync.dma_start(xg_t[:], xt[:, g * KCH * W: (g + 1) * KCH * W])
            xgs.append(xg_t)

        for g in range(G1):
            xg_t = xgs[g]
            o_g = outs.tile([128, OCH * W], dt.bfloat16)
            for o in range(OCH):
                ps = psum.tile([128, W], dt.float32)
                for c in range(KCH):
                    # lhsT = W1 chunk [128k, 128feat]; rhs = x^T [128k, W]
                    nc.tensor.matmul(
                        ps[:],
                        w1_sb[:, c * cfg.HID + o * 128: c * cfg.HID + (o + 1) * 128],
                        xg_t[:, c * W: (c + 1) * W],
                        start=(c == 0), stop=(c == KCH - 1),
                    )
                if o % 2 == 0:
                    nc.scalar.activation(o_g[:, o * W: (o + 1) * W], ps[:],
                                         mybir.ActivationFunctionType.Copy)
                else:
                    nc.vector.tensor_copy(o_g[:, o * W: (o + 1) * W], ps[:])
            nc.scalar.dma_start(h1t[:, g * OCH * W: (g + 1) * OCH * W], o_g[:])
    nc.finalize()
    return nc


def _build_mp(cfg: Cfg, plan: Plan, layer2: bool):
    """layer2: MP1 + b1 + ReLU + @(W2 Wp) -> T2. else: MP2 + bpp -> y (bf16)."""
    import concourse.bacc as bacc
    import concourse.mybir as mybir
    import concourse.tile as tile

    dt = mybir.dt
    F = cfg.HID if layer2 else cfg.OUT           # message feature width
    FCH = F // 128
    NT, TG = cfg.NTILES, cfg.TG
    tot = plan.tot
    mdt = dt.float8e3 if cfg.fp8_msg else dt.bfloat16
    full = layer2 and cfg.full_slab_l2
    GC = 32 if full else cfg.GC
    SLAB = plan.SLAB2 if full else plan.SLAB
    sdt = dt.float8e3 if full else dt.bfloat16
    nc = bacc.Bacc(None, target_bir_lowering=False)

    msg = nc.dram_tensor("msg", [128, tot * F], mdt, kind="ExternalInput")
    wsl = nc.dram_tensor("wsl", [128, SLAB], sdt, kind="ExternalInput")
    bias = nc.dram_tensor("bias", [128, F], dt.bfloat16, kind="ExternalInput")
    ident = nc.dram_tensor("ident", [128, 128], dt.bfloat16,
                           kind="ExternalInput")
    if layer2:
        wnext = nc.dram_tensor("wnext", [128, FCH * cfg.OUT], dt.bfloat16,
                               kind="ExternalInput")
    out = nc.dram_tensor("out", [128, NT * cfg.OUT], dt.bfloat16,
                         kind="ExternalOutput")

    # slab pieces: a small first piece (2 tiles) so the first matmul only
    # waits ~1 tile worth of columns, then TG-tile pieces
    cut_tiles = [0, min(2, NT)] + list(range(TG, NT, TG)) + [NT]
    cut_tiles = sorted(set(cut_tiles))
    cut_chunks = [int(plan.base[t]) for t in cut_tiles]
    if full:
        cut_cols = [c * 128 for c in cut_chunks]
    else:
        cut_cols = [int(plan.slab_off[c]) if c < tot else plan.SLAB
                    for c in cut_chunks]
    cut_cols[-1] = SLAB

    # message group boundaries: ramped small first groups, then GC chunks
    gb = [0]
    for step in (8, 16, 32):
        if gb[-1] < tot:
            gb.append(min(gb[-1] + min(step, GC), tot))
    while gb[-1] < tot:
        gb.append(min(gb[-1] + GC, tot))
    import numpy as _np
    chunk2grp = _np.searchsorted(_np.asarray(gb), _np.arange(tot),
                                 side="right") - 1

    with tile.TileContext(nc) as tc, ExitStack() as ctx:
        consts = ctx.enter_context(tc.tile_pool(name="consts", bufs=1))
        mg = ctx.enter_context(tc.tile_pool(name="mg", bufs=4))
        work = ctx.enter_context(tc.tile_pool(name="work", bufs=4))
        outs = ctx.enter_context(tc.tile_pool(name="outs", bufs=2))
        psmp = ctx.enter_context(tc.tile_pool(name="psmp", bufs=4, space="PSUM"))
        if layer2:
            pstr = ctx.enter_context(tc.tile_pool(name="pstr", bufs=2,
                                                  space="PSUM"))
            psmm = ctx.enter_context(tc.tile_pool(name="psmm", bufs=2,
                                                  space="PSUM"))

        bias_sb = consts.tile([128, F], dt.bfloat16, tag="bias")
        nc.scalar.dma_start(bias_sb[:], bias[:])
        ident_sb = consts.tile([128, 128], dt.bfloat16, tag="ident")
        nc.scalar.dma_start(ident_sb[:], ident[:])
        wsl_sb = consts.tile([128, SLAB], sdt, tag="wsl")
        nc.scalar.dma_start(wsl_sb[:, cut_cols[0]:cut_cols[1]],
                            wsl[:, cut_cols[0]:cut_cols[1]])
        if layer2:
            wnext_sb = consts.tile([128, FCH * cfg.OUT], dt.bfloat16,
                                   tag="wnext")
            nc.scalar.dma_start(wnext_sb[:], wnext[:])

        slab_state = [1]   # next piece index to issue

        def pump_slab(t):
            # keep pieces issued through tiles <= t + 2*TG
            while (slab_state[0] < len(cut_cols) - 1
                   and cut_tiles[slab_state[0]] <= t + 2 * TG):
                i = slab_state[0]
                if cut_cols[i + 1] > cut_cols[i]:
                    nc.scalar.dma_start(
                        wsl_sb[:, cut_cols[i]:cut_cols[i + 1]],
                        wsl[:, cut_cols[i]:cut_cols[i + 1]])
                slab_state[0] += 1

        gtiles = {}

        def group_tile(g):
            if g in gtiles:
                return gtiles[g]
            ck = gb[g + 1] - gb[g]
            t = mg.tile([128, GC * F], mdt)
            nc.sync.dma_start(t[:, : ck * F],
                              msg[:, gb[g] * F: gb[g + 1] * F])
            gtiles[g] = t
            return t

        o_g = None

        def tile_chunks(t):
            return [c for c in range(int(plan.base[t]), int(plan.base[t + 1]))
                    if int(plan.cM[c]) > 0]

        def chunk_mm(ps, c, last):
            if full:
                M, B, off = 128, 0, c * 128
            else:
                M = int(plan.cM[c])
                B = int(plan.cB[c])
                off = int(plan.slab_off[c])
            g = int(chunk2grp[c])
            gt = group_tile(g)
            slot = c - gb[g]
            nc.tensor.matmul(
                ps[B:B + M, :],
                wsl_sb[:, off:off + M],
                gt[:, slot * F: (slot + 1) * F],
                start=False, stop=last,
                skip_group_check=True,
            )

        def out_write(t):
            # o_g slice for tile t was filled; flush the group at boundaries
            if t % TG == TG - 1 or t == NT - 1:
                g0 = (t // TG) * TG
                nt = t - g0 + 1
                nc.scalar.dma_start(
                    out[:, g0 * cfg.OUT: (g0 + nt) * cfg.OUT],
                    o_g[:, : nt * cfg.OUT])

        def oslice_for(t):
            nonlocal o_g
            if t % TG == 0:
                o_g = outs.tile([128, TG * cfg.OUT], dt.bfloat16)
            return o_g[:, (t % TG) * cfg.OUT: (t % TG + 1) * cfg.OUT]

        if layer2:
            # software-pipelined post-processing: each tile's PE post work
            # (transposes, wnext) is deferred 1-2 tiles so the scalar relu /
            # vector copy latencies hide behind the next tile's chunk stream
            # (the PE executes its queue in order; only LDWEIGHTS reorders)
            acts = {}    # t -> act tile (awaiting transpose)
            actTs = {}   # t -> actT tile (awaiting wnext matmul)
            ps2s = {}    # t -> psum out (awaiting final copy)

            def stage1(t, ps):          # scalar: relu out of PSUM
                act = work.tile([128, F], dt.bfloat16)
                nc.scalar.activation(act[:], ps[:],
                                     mybir.ActivationFunctionType.Relu)
                acts[t] = act

            def stage2(t):              # PE: transpose + vector copy
                act = acts.pop(t)
                trp = pstr.tile([128, F], dt.bfloat16)
                for c in range(FCH):
                    nc.tensor.transpose(trp[:, c * 128:(c + 1) * 128],
                                        act[:, c * 128:(c + 1) * 128],
                                        ident_sb[:])
                actT = work.tile([128, F], dt.bfloat16)
                nc.vector.tensor_copy(actT[:], trp[:])
                actTs[t] = actT

            def stage3(t):              # PE: @ (W2 Wp)
                actT = actTs.pop(t)
                ps2 = psmm.tile([128, cfg.OUT], dt.float32)
                for c in range(FCH):
                    nc.tensor.matmul(ps2[:], actT[:, c * 128:(c + 1) * 128],
                                     wnext_sb[:, c * cfg.OUT:(c + 1) * cfg.OUT],
                                     start=(c == 0), stop=(c == FCH - 1))
                ps2s[t] = ps2

            def stage4(t):              # scalar: copy out + flush
                ps2 = ps2s.pop(t)
                nc.scalar.activation(oslice_for(t), ps2[:],
                                     mybir.ActivationFunctionType.Copy)
                out_write(t)

            for t in range(NT + 2):
                if t < NT:
                    pump_slab(t)
                    chunks = tile_chunks(t)
                    ps = psmp.tile([128, F], dt.float32)
                    nc.tensor.matmul(ps[:], ident_sb[:], bias_sb[:],
                                     start=True, stop=False,
                                     skip_group_check=True)
                    for j, c in enumerate(chunks):
                        chunk_mm(ps, c, j == len(chunks) - 1)
                    stage1(t, ps)
                if t - 1 >= 0 and t - 1 < NT:
                    stage2(t - 1)
                if t - 2 >= 0:
                    stage3(t - 2)
                    stage4(t - 2)
        else:
            for t in range(NT):
                pump_slab(t)
                chunks = tile_chunks(t)
                ps = psmp.tile([128, F], dt.float32)
                nc.tensor.matmul(ps[:], ident_sb[:], bias_sb[:],
                                 start=True, stop=False, skip_group_check=True)
                for j, c in enumerate(chunks):
                    chunk_mm(ps, c, j == len(chunks) - 1)
                oslice = oslice_for(t)
                if t % 2 == 0:
                    nc.scalar.activation(oslice, ps[:],
                                         mybir.ActivationFunctionType.Copy)
                else:
                    nc.vector.tensor_copy(oslice, ps[:])
                out_write(t)

    nc.finalize()
    return nc


# ---------------------------------------------------------------- host packing

def _quant_rows(table):
    """fp8 e3m4 per-row quantization. Returns (q [R,F] fp8, scale [R] f32)."""
    a = np.asarray(table, np.float32)
    s = np.abs(a).max(axis=1) / FP8_MAX
    s[s == 0] = 1.0
    q = (a / s[:, None]).astype(FP8)
    return q, s.astype(np.float32)


def _pack_l1_inputs(cfg: Cfg, plan: Plan, x, W1):
    KCH = cfg.IN_DIM // 128
    G1, W = cfg.G1, cfg.NG1 * 128
    w1r = np.zeros((128, KCH * cfg.HID), BF16)
    for c in range(KCH):
        w1r[:, c * cfg.HID:(c + 1) * cfg.HID] = \
            W1[c * 128:(c + 1) * 128, :].astype(BF16)
    xdt = FP8 if cfg.fp8_x else BF16
    maps = []
    for k in range(cfg.NCORES):
        xs = np.zeros((G1 * W, cfg.IN_DIM), np.float32)
        xs[:cfg.ND] = x[plan.nodes[k]]
        if cfg.fp8_x:
            # global scale; its inverse is folded into this core's W1 copy
            m = max(float(np.abs(xs).max()), 1e-30)
            xs = xs * (FP8_MAX / m)
        # [g, n, c, kk] -> [kk, g, c, n]
        xtr = np.ascontiguousarray(
            xs.reshape(G1, W, KCH, 128).transpose(3, 0, 2, 1)
        ).reshape(128, G1 * KCH * W).astype(xdt)
        if cfg.fp8_x:
            mp = {"xt": xtr, "w1": (w1r.astype(np.float32) * (m / FP8_MAX)
                                    ).astype(BF16)}
        else:
            mp = {"xt": xtr, "w1": w1r}
        maps.append(mp)
    return maps


def _unpack_h1t(cfg: Cfg, arr):
    # h1t [128, G1*OCH*W] -> H1 [NP, HID]
    OCH = cfg.HID // 128
    G1, W = cfg.G1, cfg.NG1 * 128
    a = np.asarray(arr).reshape(128, G1, OCH, W)
    # H1[g*W+n, o*128+p] = a[p, g, o, n]
    return np.ascontiguousarray(
        a.transpose(1, 3, 2, 0)).reshape(G1 * W, cfg.HID)[:cfg.NP]


def _untile_out(cfg: Cfg, arr):
    # [128, NT*OUT] -> [NP, OUT]
    return np.ascontiguousarray(
        np.asarray(arr).reshape(128, cfg.NTILES, cfg.OUT).transpose(1, 0, 2)
    ).reshape(cfg.NP, cfg.OUT)


def _pack_mp_inputs(cfg: Cfg, plan: Plan, table, Wn, b, layer2):
    F = cfg.HID if layer2 else cfg.OUT
    full = layer2 and cfg.full_slab_l2
    ident = np.eye(128, dtype=BF16)
    if cfg.fp8_msg:
        qtab, scale = _quant_rows(table)
    else:
        qtab, scale = np.asarray(table).astype(BF16), np.ones(
            table.shape[0], np.float32)
    maps = []
    for k in range(cfg.NCORES):
        gathered = qtab[plan.midx[k]]                    # [tot*128, F]
        msg = np.ascontiguousarray(
            gathered.reshape(plan.tot, 128, F).transpose(1, 0, 2)
        ).reshape(128, plan.tot * F)
        if full:
            # fp8 slab scaled by 1/alpha; fold alpha into bias and wnext:
            # relu((MP + b1)/a) @ (a*W2p) == relu(MP + b1) @ W2p  (a > 0)
            wslk, alpha = plan.build_slab_full(k, scale)
            biask = np.tile((b / alpha).astype(BF16)[None, :], (128, 1))
            wnk = Wn * alpha
        else:
            wslk = plan.build_slab(k, scale)
            biask = np.tile(b.astype(BF16)[None, :], (128, 1))
            wnk = Wn
        m = {
            "msg": msg,
            "wsl": wslk,
            "bias": biask,
            "ident": ident,
        }
        if layer2:
            FCH = cfg.HID // 128
            wnr = np.zeros((128, FCH * cfg.OUT), BF16)
            for c in range(FCH):
                wnr[:, c * cfg.OUT:(c + 1) * cfg.OUT] = \
                    wnk[c * 128:(c + 1) * 128, :].astype(BF16)
            m["wnext"] = wnr
        maps.append(m)
    return maps


# ---------------------------------------------------------------- driver

def _run(nc, in_maps, cfg, trace=False):
    from concourse.bass_utils import run_bass_kernel_spmd
    res = run_bass_kernel_spmd(nc, in_maps, list(range(cfg.NCORES)), trace=trace)
    return res


def kernel_run(inputs, cfg=None, trace=False, sim=False):
    cfg = cfg or Cfg()
    x = np.asarray(inputs["x"], np.float32)
    plan = Plan(cfg, np.asarray(inputs["edge_index"]),
                np.asarray(inputs["edge_weight"], np.float32))
    W1 = np.asarray(inputs["W1"], np.float32)
    b1 = np.asarray(inputs["b1"], np.float32)
    W2 = np.asarray(inputs["W2"], np.float32)
    b2 = np.asarray(inputs["b2"], np.float32)
    Wp = np.asarray(inputs["Wp"], np.float32)
    bp = np.asarray(inputs["bp"], np.float32)

    results = []

    def run(build, maps, outname):
        nc = build()
        if sim:
            from concourse.bass_interp import CoreSim
            outs = []
            for k in range(cfg.NCORES):
                s = CoreSim(nc)
                for name, arr in maps[k].items():
                    s.tensor(name)[:] = arr
                s.simulate()
                outs.append({outname: s.tensor(outname).copy()})
            results.append(None)
            return outs
        r = _run(nc, maps, cfg, trace=trace)
        results.append(r)
        return r.results

    # fold the post-projection into layer 2: A(relu1@W2)@Wp = A(relu1@(W2@Wp))
    W2p = (W2 @ Wp).astype(np.float32)
    bpp = (b2 @ Wp + bp).astype(np.float32)

    def asnp(a, dtype):
        a = np.asarray(a)
        return a if a.dtype == dtype else a.view(dtype)

    r1 = run(lambda: _build_l1(cfg), _pack_l1_inputs(cfg, plan, x, W1), "h1t")
    T1 = np.concatenate(
        [_unpack_h1t(cfg, asnp(r["h1t"], BF16)) for r in r1], axis=0)

    r2 = run(lambda: _build_mp(cfg, plan, True),
             _pack_mp_inputs(cfg, plan, T1, W2p, b1, True), "out")
    T2 = np.concatenate(
        [_untile_out(cfg, asnp(r["out"], BF16)) for r in r2], axis=0)

    r3 = run(lambda: _build_mp(cfg, plan, False),
             _pack_mp_inputs(cfg, plan, T2, None, bpp, False), "out")

    y = np.empty((cfg.N, cfg.OUT), np.float32)
    for k in range(cfg.NCORES):
        shard = _untile_out(cfg, asnp(r3[k]["out"], BF16)).astype(np.float32)
        y[plan.nodes[k]] = shard[:cfg.ND]
    return y, results


def kernel(**inputs):
    y, _ = kernel_run(inputs)
    return y
